# revision 7
# baseline (speedup 1.0000x reference)
"""Trainium2 Bass kernel for nn_MultiHead_68624987456278.

GQA multi-head attention layer (RoPE, causal softmax, output projection)
  B=4, T=2048, C=2048, 16 q-heads / 4 kv-heads, d_k=128.

Sharding (8 cores): data-parallel over batch (4) x tensor-parallel over
head-halves (2).  Core i handles batch b=i//2 and head-half h=i%2
(q-heads 8h..8h+7, kv-heads 2h,2h+1).  Each core computes a partial
output  out_partial = Attn_half(x_b) @ Wp[rows of its heads, :]  and the
host sums the two partials per batch (+ bias).  No device collectives.

Per-core pipeline (all matmuls bf16 inputs, fp32 PSUM accumulation):
  A) QKV projection from resident xT (bf16) with weights streamed;
     RoPE applied in [d, t] layout via stream_shuffle pair-swap.
  B) Attention per q-head in transposed-score layout:
     S_T[tk,tq] = K_chunk^T-style matmul, P=exp(S/sqrt(d)) on ScalarE,
     causal diag-masking via bf16 multiply, O_T accum + row-sums via
     ones-matmul, normalization via reciprocal + partition_broadcast.
  C) Output projection O_T @ Wp -> partial [T, C] fp32.
"""

import sys

sys.path.insert(0, "/opt/trn_rl_repo")

import numpy as np
import ml_dtypes
from contextlib import ExitStack

import concourse.bass as bass  # noqa: F401  (import keeps bass registered)
import concourse.tile as tile
from concourse import bacc, mybir
from concourse import bass_utils

BF16 = mybir.dt.bfloat16
F32 = mybir.dt.float32
P = 128
SWAP_MASK = [i ^ 1 for i in range(32)]  # pair swap within 32-partition quadrant
EXP = mybir.ActivationFunctionType.Exp


def emit_core_kernel(tc, io, T=2048, C=2048, NQ=8, NKV=2, G=4):
    """Emit the per-core program. io: dict of dram APs."""
    nc = tc.nc
    NU = NQ + NKV
    NT4 = T // 512  # tq tiles of 512
    NCC = C // P  # contraction chunks over C
    NTCH = T // P  # t chunks of 128
    NYB = C // 512  # output col blocks
    sc = 128.0**-0.5

    with ExitStack() as stk0:
        const = stk0.enter_context(tc.tile_pool(name="const", bufs=1))
        qk_pool = stk0.enter_context(tc.tile_pool(name="qk", bufs=NU))
        v_pool = stk0.enter_context(tc.tile_pool(name="vsb", bufs=NTCH))
        o_pool = stk0.enter_context(tc.tile_pool(name="osb", bufs=NQ))

        cc_sb = const.tile([P, T], BF16, tag="cc")
        nc.sync.dma_start(cc_sb, io["cc"])
        ss_sb = const.tile([P, T], BF16, tag="ss")
        nc.sync.dma_start(ss_sb, io["ss"])
        mk_sb = const.tile([P, 4, 512], BF16, tag="mk")
        nc.sync.dma_start(mk_sb, io["mk"])
        ones_sb = const.tile([P, 1], BF16, tag="ones")
        nc.vector.memset(ones_sb, 1.0)

        k_sb = []
        q_sb = []
        o_sb = [
            o_pool.tile([P, T], BF16, tag="osb", name=f"osb{j}") for j in range(NQ)
        ]
        v_sb = []

        stk1 = ExitStack()
        xt_pool = stk1.enter_context(tc.tile_pool(name="xt", bufs=NCC))
        w_pool = stk1.enter_context(tc.tile_pool(name="w", bufs=2))
        rp = stk1.enter_context(tc.tile_pool(name="rope", bufs=2))
        psA = stk1.enter_context(tc.tile_pool(name="psA", bufs=2, space="PSUM"))

        xt = []
        for c in range(NCC):
            t = xt_pool.tile([P, T], BF16, tag="xtt")
            nc.sync.dma_start(t, io["xT"][c * P : (c + 1) * P, :])
            xt.append(t)

        def project_unit(u, dst):
            """dst[:, :] = RoPE((x @ Wu).T) in [d, t] layout, bf16."""
            wu = w_pool.tile([P, NCC, 128], BF16, tag="wu")
            nc.sync.dma_start(wu, io["wqk"][:, :, u * 128 : (u + 1) * 128])
            for t4 in range(NT4):
                tsl = slice(t4 * 512, (t4 + 1) * 512)
                y = psA.tile([P, 512], F32, tag="psA")
                for c in range(NCC):
                    nc.tensor.matmul(
                        y,
                        lhsT=wu[:, c, :],
                        rhs=xt[c][:, tsl],
                        start=(c == 0),
                        stop=(c == NCC - 1),
                    )
                ysw = rp.tile([P, 512], F32, tag="ysw")
                nc.vector.stream_shuffle(ysw, y, mask=SWAP_MASK)
                t1 = rp.tile([P, 512], F32, tag="t1")
                nc.vector.tensor_mul(t1, y, cc_sb[:, tsl])
                t2 = rp.tile([P, 512], BF16, tag="t2")
                nc.vector.tensor_mul(t2, ysw, ss_sb[:, tsl])
                nc.vector.tensor_add(dst[:, tsl], t1, t2)

        # K units first so attention can start as soon as each q head is done.
        for u in range(NKV):
            dst = qk_pool.tile([P, T], BF16, tag="qk")
            k_sb.append(dst)
            project_unit(u, dst)

        # V block: V in [t, d] layout (lhsT = xT chunk).
        with ExitStack() as stk2:
            wv_pool = stk2.enter_context(tc.tile_pool(name="wv", bufs=1))
            psV = stk2.enter_context(tc.tile_pool(name="psV", bufs=2, space="PSUM"))
            wvt = wv_pool.tile([P, NCC, NKV * 128], BF16, tag="wvt")
            nc.sync.dma_start(wvt, io["wv"])
            for ti in range(NTCH):
                yv = psV.tile([P, NKV * 128], F32, tag="psV")
                for c in range(NCC):
                    nc.tensor.matmul(
                        yv,
                        lhsT=xt[c][:, ti * P : (ti + 1) * P],
                        rhs=wvt[:, c, :],
                        start=(c == 0),
                        stop=(c == NCC - 1),
                    )
                vt = v_pool.tile([P, NKV * 128], BF16, tag="vt")
                nc.scalar.copy(vt, yv)
                v_sb.append(vt)

        # Attention pools (PSUM budget: psA2 + psS2 + psO2 + psSum2 = 8 banks)
        stk3 = ExitStack()
        p_pool = stk3.enter_context(tc.tile_pool(name="pp", bufs=6))
        rc_pool = stk3.enter_context(tc.tile_pool(name="rc", bufs=2))
        rb_pool = stk3.enter_context(tc.tile_pool(name="rb", bufs=2))
        psS = stk3.enter_context(tc.tile_pool(name="psS", bufs=2, space="PSUM"))
        psO = stk3.enter_context(tc.tile_pool(name="psO", bufs=2, space="PSUM"))
        psSum = stk3.enter_context(tc.tile_pool(name="psSum", bufs=2, space="PSUM"))

        for j in range(NQ):
            dst = qk_pool.tile([P, T], BF16, tag="qk")
            q_sb.append(dst)
            project_unit(NKV + j, dst)
            n = j // G
            for q4 in range(NT4):
                qsl = slice(q4 * 512, (q4 + 1) * 512)
                o_ps = psO.tile([P, 512], F32, tag="psO")
                s_ps = psSum.tile([1, 512], F32, tag="psSum")
                nch = 4 * (q4 + 1)
                for c in range(nch):
                    S_ps = psS.tile([P, 512], F32, tag="psS")
                    nc.tensor.matmul(
                        S_ps,
                        lhsT=k_sb[n][:, c * P : (c + 1) * P],
                        rhs=q_sb[j][:, qsl],
                        start=True,
                        stop=True,
                        skip_group_check=True,
                    )
                    pt = p_pool.tile([P, 512], BF16, tag="pt")
                    nc.scalar.activation(pt, S_ps, EXP, scale=sc)
                    if c >= 4 * q4:
                        nc.vector.tensor_mul(pt, pt, mk_sb[:, c - 4 * q4, :])
                    nc.tensor.matmul(
                        o_ps,
                        lhsT=v_sb[c][:, n * 128 : (n + 1) * 128],
                        rhs=pt,
                        start=(c == 0),
                        stop=(c == nch - 1),
                        skip_group_check=True,
                    )
                    nc.tensor.matmul(
                        s_ps,
                        lhsT=ones_sb,
                        rhs=pt,
                        start=(c == 0),
                        stop=(c == nch - 1),
                        skip_group_check=True,
                    )
                rc = rc_pool.tile([1, 512], F32, tag="rc")
                nc.vector.reciprocal(rc, s_ps)
                rb = rb_pool.tile([P, 512], F32, tag="rb")
                nc.gpsimd.partition_broadcast(rb, rc)
                nc.vector.tensor_mul(o_sb[j][:, qsl], o_ps, rb)

        stk3.close()
        stk1.close()

        # Phase C: out_partial[t, y] = sum_j O_T[j].T @ Wp[j]
        with ExitStack() as stk4:
            wp_pool = stk4.enter_context(tc.tile_pool(name="wp", bufs=NQ))
            outc = stk4.enter_context(tc.tile_pool(name="outc", bufs=3))
            psC = stk4.enter_context(tc.tile_pool(name="psC", bufs=3, space="PSUM"))
            wp_sb = []
            for j in range(NQ):
                w = wp_pool.tile([P, C], BF16, tag="wp")
                nc.sync.dma_start(w, io["wp"][j * P : (j + 1) * P, :])
                wp_sb.append(w)
            for m in range(NTCH):
                msl = slice(m * P, (m + 1) * P)
                for nb in range(NYB):
                    ysl = slice(nb * 512, (nb + 1) * 512)
                    py = psC.tile([P, 512], F32, tag="psC")
                    for j in range(NQ):
                        nc.tensor.matmul(
                            py,
                            lhsT=o_sb[j][:, msl],
                            rhs=wp_sb[j][:, ysl],
                            start=(j == 0),
                            stop=(j == NQ - 1),
                        )
                    ot = outc.tile([P, 512], F32, tag="ot")
                    nc.scalar.copy(ot, py)
                    nc.sync.dma_start(io["out"][msl, ysl], ot)


def build_program(T=2048, C=2048, NQ=8, NKV=2, G=4):
    nc = bacc.Bacc("TRN2", target_bir_lowering=False, debug=False)
    NU = NQ + NKV
    NCC = C // P
    io = {
        "xT": nc.dram_tensor("xT", [C, T], BF16, kind="ExternalInput").ap(),
        "wqk": nc.dram_tensor(
            "wqk", [P, NCC, NU * 128], BF16, kind="ExternalInput"
        ).ap(),
        "wv": nc.dram_tensor("wv", [P, NCC, NKV * 128], BF16, kind="ExternalInput").ap(),
        "wp": nc.dram_tensor("wp", [NQ * P, C], BF16, kind="ExternalInput").ap(),
        "cc": nc.dram_tensor("cc", [P, T], BF16, kind="ExternalInput").ap(),
        "ss": nc.dram_tensor("ss", [P, T], BF16, kind="ExternalInput").ap(),
        "mk": nc.dram_tensor("mk", [P, 4, 512], BF16, kind="ExternalInput").ap(),
        "out": nc.dram_tensor("out", [T, C], F32, kind="ExternalOutput").ap(),
    }
    with tile.TileContext(nc) as tc:
        emit_core_kernel(tc, io, T=T, C=C, NQ=NQ, NKV=NKV, G=G)
    nc.compile()
    return nc


def make_tables(T):
    """RoPE tables in [d, t] layout + causal diag masks, fp32."""
    theta = 10000.0 ** (-2.0 * np.arange(0, 128, 2, dtype=np.float64) / 128.0)
    freq = np.arange(T, dtype=np.float64)[None, :] * theta[:, None]  # [64, T]
    cos = np.cos(freq).astype(np.float32)
    sin = np.sin(freq).astype(np.float32)
    cc = np.repeat(cos, 2, axis=0)  # [128, T]
    ss = np.repeat(sin, 2, axis=0)
    ss[0::2, :] *= -1.0
    mk = np.zeros((P, 4, 512), np.float32)
    tk = np.arange(P)[:, None]
    tq = np.arange(512)[None, :]
    for jj in range(4):
        mk[:, jj, :] = (tk + 128 * jj <= tq).astype(np.float32)
    return cc, ss, mk


_PROGRAM = None


def _get_program():
    global _PROGRAM
    if _PROGRAM is None:
        _PROGRAM = build_program()
    return _PROGRAM


def prepare_in_maps(x, Wq, Wk, Wv, Wp):
    x = np.asarray(x, np.float32)
    Wq = np.asarray(Wq, np.float32)
    Wk = np.asarray(Wk, np.float32)
    Wv = np.asarray(Wv, np.float32)
    Wp = np.asarray(Wp, np.float32)
    B, T, C = x.shape
    bf = ml_dtypes.bfloat16
    NCC = C // P

    cc, ss, mk = make_tables(T)
    cc = cc.astype(bf)
    ss = ss.astype(bf)
    mk = mk.astype(bf)

    in_maps = []
    for core in range(8):
        b, h = core // 2, core % 2
        xT = np.ascontiguousarray(x[b].T).astype(bf)
        # units: kv heads {2h, 2h+1} then q heads {8h..8h+7}
        wqk = np.concatenate(
            [Wk[:, h * 256 : (h + 1) * 256], Wq[:, h * 1024 : (h + 1) * 1024]], axis=1
        )  # [C, 1280]
        wqk_r = np.ascontiguousarray(
            wqk.reshape(NCC, P, 1280).transpose(1, 0, 2)
        ).astype(bf)
        wv_r = np.ascontiguousarray(
            Wv[:, h * 256 : (h + 1) * 256].reshape(NCC, P, 256).transpose(1, 0, 2)
        ).astype(bf)
        wp_l = np.ascontiguousarray(Wp[h * 1024 : (h + 1) * 1024, :]).astype(bf)
        in_maps.append(
            {
                "xT": xT,
                "wqk": wqk_r,
                "wv": wv_r,
                "wp": wp_l,
                "cc": cc,
                "ss": ss,
                "mk": mk,
            }
        )

    return in_maps


def gather_output(results, bp, B=4, T=2048, C=2048):
    bp = np.asarray(bp, np.float32)
    parts = [np.asarray(results[i]["out"], np.float32) for i in range(8)]
    out = np.empty((B, T, C), np.float32)
    for b in range(B):
        out[b] = parts[2 * b] + parts[2 * b + 1] + bp[None, :]
    return out


def kernel(x, Wq, Wk, Wv, Wp, bp):
    B, T, C = np.asarray(x).shape
    in_maps = prepare_in_maps(x, Wq, Wk, Wv, Wp)
    nc = _get_program()
    res = bass_utils.run_bass_kernel_spmd(nc, in_maps, core_ids=list(range(8)))
    return gather_output(res.results, bp, B=B, T=T, C=C)
